# revision 31
# baseline (speedup 1.0000x reference)
"""Bass/Trainium2 kernel for nn_BakaAttention: 8-way data-parallel over batch.

Per core (one batch element):
  q = rope(x@wq, off=1024); k = concat(rope_host(past_k), rope(x@wk));
  v = concat(past_v, x@wv); out = softmax(mask(q k^T / 16)) v @ wo

All matmuls run in bf16 (1 cycle/row on the PE vs 2 for f32r's fp32-HIGH
mode). x is pre-transposed and past_k pre-roped+transposed on the host, so
the device does no transposes. Everything stays SBUF-resident (no DRAM
spills). Scores are computed transposed [keys, queries] so PV consumes the
probs directly as the moving operand; softmax row-sums accumulate via a
ones-column matmul and the normalization uses the fast custom-DVE
reciprocal off the critical path.
"""

import numpy as np

B, T, P, H, DH, DIN, DOUT = 8, 1024, 1024, 4, 256, 1024, 1152
S = P + T  # 2048 keys
THETA = 10000.0
NCORES = 8


def _host_prep(x, past_k, past_v, wq, wk, wv, wo):
    """Per-batch input prep: bf16 casts, transposes, past_k rope."""
    import ml_dtypes

    bf16 = ml_dtypes.bfloat16
    x, past_k, past_v, wq, wk, wv, wo = (
        np.asarray(a) for a in (x, past_k, past_v, wq, wk, wv, wo))

    m = np.arange(0, DH, 2, dtype=np.float64) / DH          # 128 freqs
    inv = 1.0 / (THETA ** m)                                # [128]

    # past_k rope at offset 0, interleaved pairs
    pos = np.arange(P, dtype=np.float64)
    ang = np.outer(pos, inv)                                # [P, 128]
    c = np.cos(ang)[:, None, :]                             # [P, 1, 128]
    s = np.sin(ang)[:, None, :]
    pk = past_k.astype(np.float64)                          # [B, P, H, DH]
    x1, x2 = pk[..., 0::2], pk[..., 1::2]
    o1 = x1 * c - x2 * s
    o2 = x2 * c + x1 * s
    pk_rot = np.stack([o1, o2], axis=-1).reshape(B, P, H, DH)
    # kT layout: [B, 8 ftile, 128, P]; ftile ft=2h+f2 covers head-local
    # features 128*f2 + p
    pkT = np.ascontiguousarray(
        pk_rot.reshape(B, P, 8, 128).transpose(0, 2, 3, 1)
    ).astype(bf16)

    # device rope tables for positions P..P+T-1 (used by both q and new-k)
    posq = np.arange(P, P + T, dtype=np.float64)
    angq = np.outer(inv, posq)                              # [128 m, T]
    cosq, sinq = np.cos(angq), np.sin(angq)
    r = np.arange(128) // 2
    tabs = np.ascontiguousarray(np.stack(
        [cosq[r, :], cosq[64 + r, :], sinq[r, :], sinq[64 + r, :]],
        axis=1)).astype(bf16)  # [128, 4, T]

    # pair-rotation matrix: rot = prot.T @ raw -> rot[2m] = -raw[2m+1],
    # rot[2m+1] = raw[2m]
    prot = np.zeros((128, 128), np.float32)
    for mm in range(64):
        prot[2 * mm, 2 * mm + 1] = 1.0
        prot[2 * mm + 1, 2 * mm] = -1.0

    # masks[p, ci, tl] = 1 if key p within diag block ci is visible to
    # local query tl
    sl = np.arange(128)[:, None]
    tl = np.arange(512)[None, :]
    masks = np.stack(
        [(sl <= tl - 128 * ci).astype(np.float32) for ci in range(4)], axis=1
    )

    common = {
        "wqp": np.ascontiguousarray(
            wq.reshape(8, 128, 4, 256).transpose(2, 0, 1, 3)).astype(bf16),
        "wkp": np.ascontiguousarray(
            wk.reshape(8, 128, 4, 256).transpose(2, 0, 1, 3)).astype(bf16),
        "wv": wv.reshape(8, 128, DIN).astype(bf16),
        "wo": wo.reshape(8, 128, DOUT).astype(bf16),
        "masks": np.ascontiguousarray(masks).astype(bf16),
        "prot": prot.astype(bf16),
        "ones": np.ones((128, 1), bf16),
        "tabs": tabs,
    }

    per_core = []
    for b in range(NCORES):
        per_core.append({
            "xT": np.ascontiguousarray(x[b].T).astype(bf16),
            "pkT": pkT[b],
            "pvf": np.ascontiguousarray(
                past_v[b].reshape(P, DIN).reshape(8, 128, DIN)).astype(bf16),
            **common,
        })
    return per_core


def build_kernel():
    import concourse.bass as bass  # noqa: F401
    import concourse.mybir as mybir
    from concourse import bacc
    from concourse.tile import TileContext

    f32 = mybir.dt.float32
    bf = mybir.dt.bfloat16
    AF = mybir.ActivationFunctionType
    OP = mybir.AluOpType

    nc = bacc.Bacc(None, target_bir_lowering=False)

    xT_d = nc.dram_tensor("xT", [DIN, T], bf, kind="ExternalInput")
    wqp_d = nc.dram_tensor("wqp", [4, 8, 128, 256], bf, kind="ExternalInput")
    wkp_d = nc.dram_tensor("wkp", [4, 8, 128, 256], bf, kind="ExternalInput")
    wv_d = nc.dram_tensor("wv", [8, 128, DIN], bf, kind="ExternalInput")
    wo_d = nc.dram_tensor("wo", [8, 128, DOUT], bf, kind="ExternalInput")
    pkT_d = nc.dram_tensor("pkT", [8, 128, P], bf, kind="ExternalInput")
    pvf_d = nc.dram_tensor("pvf", [8, 128, DIN], bf, kind="ExternalInput")
    tabs_d = nc.dram_tensor("tabs", [128, 4, T], bf, kind="ExternalInput")
    prot_d = nc.dram_tensor("prot", [128, 128], bf, kind="ExternalInput")
    masks_d = nc.dram_tensor("masks", [128, 4, 512], bf, kind="ExternalInput")
    ones_d = nc.dram_tensor("ones", [128, 1], bf, kind="ExternalInput")
    out_d = nc.dram_tensor("out", [T, DOUT], f32, kind="ExternalOutput")

    from contextlib import ExitStack
    stack = ExitStack()
    with TileContext(nc) as tc, stack:
        cst = stack.enter_context(tc.tile_pool(name="consts", bufs=1))
        dat = stack.enter_context(tc.tile_pool(name="data", bufs=1))
        wtp = stack.enter_context(tc.tile_pool(name="wt", bufs=1))
        rawp = stack.enter_context(tc.tile_pool(name="raw", bufs=4))
        pjp = stack.enter_context(tc.tile_pool(name="pj", bufs=6))
        smlp = stack.enter_context(tc.tile_pool(name="sml", bufs=2))
        otp = stack.enter_context(tc.tile_pool(name="ot", bufs=3))
        quad = stack.enter_context(tc.tile_pool(name="quad", bufs=3, space="PSUM"))
        mxp = stack.enter_context(tc.tile_pool(name="mx", bufs=2, space="PSUM"))
        ytpp = stack.enter_context(tc.tile_pool(name="ytp", bufs=1, space="PSUM"))
        smbc = stack.enter_context(tc.tile_pool(name="smbc", bufs=1, space="PSUM"))

        # ---- persistent SBUF tiles + startup DMAs ----
        # Emission order = SP issue order; the first projection matmuls need
        # wq(h0)[kt] + xT[kt], so those lead, interleaved, with the rope
        # constants woven in. Everything else follows in order of first use.
        # xT in column halves: the first projection pass needs only the
        # th=0 half, halving the startup DMA footprint
        xTh = [[dat.tile([128, 512], bf, name=f"xT{i}_{hf}", tag=f"xT{i}_{hf}")
                for hf in range(2)] for i in range(8)]
        tabs = cst.tile([128, 4, T], bf, name="tabs", tag="tabs")
        prot = cst.tile([128, 128], bf, name="prot", tag="prot")
        ones_sb = cst.tile([128, 1], bf, name="ones", tag="ones")
        masks = cst.tile([128, 4, 512], bf, name="masks", tag="masks")
        kT = [dat.tile([128, S], bf, name=f"kT{i}", tag=f"kT{i}") for i in range(8)]
        pv_sb = [dat.tile([128, DIN], bf, name=f"pv{i}", tag=f"pv{i}")
                 for i in range(8)]
        qT = [dat.tile([128, T], bf, name=f"qT{i}", tag=f"qT{i}") for i in range(8)]

        # Startup DMA issue is spread over three queues so the projection
        # pipeline is fed at matmul pace: SP carries the q-weights (+ the
        # rest of the input stream), GpSimd carries xT/pkT, ACT the small
        # rope constants.
        # PE warmup: ~32 back-to-back matmuls on a zeroed scratch tile keep
        # the HAM clock-gate warm through the DMA-gated startup window
        scratch = wtp.tile([128, 512], bf, name="warm", tag="warm")
        nc.vector.memset(scratch[:], 0.0)
        warm_ps = quad.tile([128, 512], f32, name="warm_ps", tag="q")
        for i in range(24):
            nc.tensor.matmul(warm_ps[:], scratch[:, 0:128], scratch[:],
                             start=(i == 0), stop=(i == 23))

        wts_q0 = [wtp.tile([128, 256], bf, name="wqk", tag=f"wqk{kt}", bufs=2)
                  for kt in range(8)]
        for kt in range(8):
            nc.sync.dma_start(out=wts_q0[kt][:], in_=wqp_d[0, kt])
            nc.gpsimd.dma_start(
                out=xTh[kt][0][:], in_=xT_d[128 * kt:128 * (kt + 1), 0:512])
        for kt in range(8):
            nc.gpsimd.dma_start(
                out=xTh[kt][1][:], in_=xT_d[128 * kt:128 * (kt + 1), 512:1024])
        nc.scalar.dma_start(out=tabs[:], in_=tabs_d[:])
        nc.scalar.dma_start(out=prot[:], in_=prot_d[:])
        nc.scalar.dma_start(out=ones_sb[:], in_=ones_d[:])
        for ft in range(8):
            nc.gpsimd.dma_start(out=kT[ft][:, 0:P], in_=pkT_d[ft])
        v_sb = [dat.tile([128, DIN], bf, name=f"v{i}", tag=f"v{i}")
                for i in range(8)]
        yT = [dat.tile([128, T], bf, name=f"yT{i}", tag=f"yT{i}") for i in range(8)]

        # ---- building blocks ----
        def rope_combine(dst_ap, raw_sb, rot_ps, f2, t0):
            # dst = raw * cos + rot * sin, tables sliced at local t0
            ctab = tabs[:, f2, t0:t0 + 512]
            stab = tabs[:, 2 + f2, t0:t0 + 512]
            t1 = rawp.tile([128, 512], bf, name="ropet1", tag="ropet1")
            nc.gpsimd.tensor_tensor(t1[:], raw_sb, ctab, op=OP.mult)
            t2 = rawp.tile([128, 512], bf, name="ropet2", tag="ropet2")
            nc.vector.tensor_tensor(t2[:], rot_ps, stab, op=OP.mult)
            nc.vector.tensor_tensor(dst_ap, t1[:], t2[:], op=OP.add)

        def proj_qk(h, w_d, dst, dst_off, wts=None):
            # dst[2h+f2][:, dst_off + t] = rope(x @ w)[128f2+p, t]
            if wts is None:
                wts = [wtp.tile([128, 256], bf, name="wqk", tag=f"wqk{kt}",
                                bufs=2) for kt in range(8)]
                for kt in range(8):
                    nc.sync.dma_start(out=wts[kt][:], in_=w_d[h, kt])
            for th in range(2):
                for f2 in range(2):
                    psl = quad.tile([128, 512], f32, name="psl", tag="q")
                    for kt in range(8):
                        nc.tensor.matmul(
                            psl[:],
                            wts[kt][:, 128 * f2:128 * (f2 + 1)],
                            xTh[kt][th][:],
                            start=(kt == 0), stop=(kt == 7))
                    raw = rawp.tile([128, 512], bf, name="raw", tag="raw")
                    nc.scalar.copy(raw[:], psl[:])
                    rp = mxp.tile([128, 512], f32, name="rot", tag="mx")
                    nc.tensor.matmul(rp[:], prot[:], raw[:],
                                     start=True, stop=True)
                    rope_combine(
                        dst[2 * h + f2][:, dst_off + 512 * th:
                                        dst_off + 512 * (th + 1)],
                        raw[:], rp[:], f2, 512 * th)

        def proj_v():
            wvs = [wtp.tile([128, DIN], bf, name="wv", tag=f"wv{kt}")
                   for kt in range(8)]
            for kt in range(8):
                nc.sync.dma_start(out=wvs[kt][:], in_=wv_d[kt])
            for st in range(8):
                for fh in range(2):
                    pv_ps = quad.tile([128, 512], f32, name="pvps", tag="q")
                    for kt in range(8):
                        nc.tensor.matmul(
                            pv_ps[:],
                            xTh[kt][st // 4][:, 128 * (st % 4):
                                             128 * (st % 4 + 1)],
                            wvs[kt][:, 512 * fh:512 * (fh + 1)],
                            start=(kt == 0), stop=(kt == 7))
                    nc.vector.tensor_copy(
                        v_sb[st][:, 512 * fh:512 * (fh + 1)], pv_ps[:])

        def vsrc(j, h, fb):
            src = pv_sb[j] if j < 8 else v_sb[j - 8]
            c0 = 256 * h + 128 * fb
            return src[:, c0:c0 + 128]

        def attention(h):
            for TH in range(2):
                jmax = 12 + 4 * TH
                ytp = [ytpp.tile([128, 512], f32, name=f"ytp{i}", tag=f"y{i}")
                       for i in range(2)]
                sm = smbc.tile([128, 512], f32, name="sm", tag="sb")
                for j in range(jmax):
                    # diagonal block ci only serves local queries >= 128*ci
                    ci = j - (8 + 4 * TH)
                    q0 = 128 * ci if ci > 0 else 0
                    sc = mxp.tile([128, 512], f32, name="sc", tag="mx")
                    for fk in range(2):
                        nc.tensor.matmul(
                            sc[:, q0:512],
                            kT[2 * h + fk][:, 128 * j:128 * (j + 1)],
                            qT[2 * h + fk][:, 512 * TH + q0:512 * (TH + 1)],
                            start=(fk == 0), stop=(fk == 1))
                    pj = pjp.tile([128, 512], bf, name="pj", tag="pj")
                    nc.scalar.activation(pj[:, q0:512], sc[:, q0:512], AF.Exp,
                                         scale=float(DH ** -0.5))
                    if ci >= 0:
                        nc.vector.tensor_tensor(
                            pj[:, q0:q0 + 128], pj[:, q0:q0 + 128],
                            masks[:, ci, q0:q0 + 128], op=OP.mult)
                    for fb in range(2):
                        nc.tensor.matmul(ytp[fb][:, q0:512],
                                         vsrc(j, h, fb), pj[:, q0:512],
                                         start=(j == 0), stop=(j == jmax - 1))
                    nc.tensor.matmul(sm[0:1, q0:512], ones_sb[:],
                                     pj[:, q0:512],
                                     start=(j == 0), stop=(j == jmax - 1))
                rc = smlp.tile([1, 512], f32, name="rc", tag="rc")
                nc.vector.reciprocal_approx_fast(out=rc[:], in_=sm[0:1, :])
                bc_sb = smlp.tile([128, 512], f32, name="bcsb", tag="bcsb")
                nc.gpsimd.partition_broadcast(bc_sb[:], rc[:])
                for fb in range(2):
                    nc.vector.tensor_tensor(
                        yT[2 * h + fb][:, 512 * TH:512 * (TH + 1)],
                        ytp[fb][:], bc_sb[:], op=OP.mult)

        # ---- program ----
        proj_qk(0, wqp_d, qT, 0, wts=wts_q0)
        proj_qk(0, wkp_d, kT, P)
        proj_v()
        for st in range(8):
            nc.sync.dma_start(out=pv_sb[st][:], in_=pvf_d[st])
        nc.sync.dma_start(out=masks[:], in_=masks_d[:])
        attention(0)
        wo_sb = [dat.tile([128, DOUT], bf, name=f"wo{i}", tag=f"wo{i}")
                 for i in range(8)]
        for h in range(1, 4):
            proj_qk(h, wqp_d, qT, 0)
            proj_qk(h, wkp_d, kT, P)
            if h == 2:
                for kt in range(8):
                    nc.sync.dma_start(out=wo_sb[kt][:], in_=wo_d[kt])
            attention(h)

        for tt in range(8):
            for ds in range(3):
                op_ps = quad.tile([128, 512], f32, name="ops", tag="q")
                for fk in range(8):
                    nc.tensor.matmul(
                        op_ps[:, 0:384],
                        yT[fk][:, 128 * tt:128 * (tt + 1)],
                        wo_sb[fk][:, 384 * ds:384 * (ds + 1)],
                        start=(fk == 0), stop=(fk == 7))
                ot = otp.tile([128, 384], f32, name="ot", tag="ot")
                nc.vector.tensor_copy(ot[:], op_ps[:, 0:384])
                nc.sync.dma_start(
                    out=out_d[128 * tt:128 * (tt + 1), 384 * ds:384 * (ds + 1)],
                    in_=ot[:])

    nc.finalize()
    return nc


_NC_CACHE = {}


def run(x, past_k, past_v, wq, wk, wv, wo, debug=False, trace=False):
    from concourse.bass_utils import run_bass_kernel_spmd

    if "nc" not in _NC_CACHE:
        _NC_CACHE["nc"] = build_kernel()
    nc = _NC_CACHE["nc"]
    in_maps = _host_prep(x, past_k, past_v, wq, wk, wv, wo)
    res = run_bass_kernel_spmd(nc, in_maps, list(range(NCORES)), trace=trace)
    out = np.stack([res.results[b]["out"] for b in range(NCORES)], axis=0)
    return out.astype(np.float32), res


def kernel(x, past_k, past_v, wq, wk, wv, wo):
    out, _ = run(x, past_k, past_v, wq, wk, wv, wo)
    return out


# revision 32
# speedup vs baseline: 1.1559x; 1.1559x over previous
"""Bass/Trainium2 kernel for nn_BakaAttention: 8-way data-parallel over batch.

Per core (one batch element):
  q = rope(x@wq, off=1024); k = concat(rope_host(past_k), rope(x@wk));
  v = concat(past_v, x@wv); out = softmax(mask(q k^T / 16)) v @ wo

All matmuls run in bf16 (1 cycle/row on the PE vs 2 for f32r's fp32-HIGH
mode). x is pre-transposed and past_k pre-roped+transposed on the host, so
the device does no transposes. Everything stays SBUF-resident (no DRAM
spills). Scores are computed transposed [keys, queries] so PV consumes the
probs directly as the moving operand; softmax row-sums accumulate via a
ones-column matmul and the normalization uses the fast custom-DVE
reciprocal off the critical path.
"""

import numpy as np

B, T, P, H, DH, DIN, DOUT = 8, 1024, 1024, 4, 256, 1024, 1152
S = P + T  # 2048 keys
THETA = 10000.0
NCORES = 8


def _host_prep(x, past_k, past_v, wq, wk, wv, wo):
    """Per-batch input prep: bf16 casts, transposes, past_k rope."""
    import ml_dtypes

    bf16 = ml_dtypes.bfloat16
    x, past_k, past_v, wq, wk, wv, wo = (
        np.asarray(a) for a in (x, past_k, past_v, wq, wk, wv, wo))

    m = np.arange(0, DH, 2, dtype=np.float64) / DH          # 128 freqs
    inv = 1.0 / (THETA ** m)                                # [128]

    # past_k rope at offset 0, interleaved pairs
    pos = np.arange(P, dtype=np.float64)
    ang = np.outer(pos, inv)                                # [P, 128]
    c = np.cos(ang)[:, None, :]                             # [P, 1, 128]
    s = np.sin(ang)[:, None, :]
    pk = past_k.astype(np.float64)                          # [B, P, H, DH]
    x1, x2 = pk[..., 0::2], pk[..., 1::2]
    o1 = x1 * c - x2 * s
    o2 = x2 * c + x1 * s
    pk_rot = np.stack([o1, o2], axis=-1).reshape(B, P, H, DH)
    # kT layout: [B, 8 ftile, 128, P]; ftile ft=2h+f2 covers head-local
    # features 128*f2 + p
    pkT = np.ascontiguousarray(
        pk_rot.reshape(B, P, 8, 128).transpose(0, 2, 3, 1)
    ).astype(bf16)

    # device rope tables for positions P..P+T-1 (used by both q and new-k)
    posq = np.arange(P, P + T, dtype=np.float64)
    angq = np.outer(inv, posq)                              # [128 m, T]
    cosq, sinq = np.cos(angq), np.sin(angq)
    r = np.arange(128) // 2
    tabs = np.ascontiguousarray(np.stack(
        [cosq[r, :], cosq[64 + r, :], sinq[r, :], sinq[64 + r, :]],
        axis=1)).astype(bf16)  # [128, 4, T]

    # pair-rotation matrix: rot = prot.T @ raw -> rot[2m] = -raw[2m+1],
    # rot[2m+1] = raw[2m]
    prot = np.zeros((128, 128), np.float32)
    for mm in range(64):
        prot[2 * mm, 2 * mm + 1] = 1.0
        prot[2 * mm + 1, 2 * mm] = -1.0

    # masks[p, ci, tl] = 1 if key p within diag block ci is visible to
    # local query tl
    sl = np.arange(128)[:, None]
    tl = np.arange(512)[None, :]
    masks = np.stack(
        [(sl <= tl - 128 * ci).astype(np.float32) for ci in range(4)], axis=1
    )

    common = {
        "wqp": np.ascontiguousarray(
            wq.reshape(8, 128, 4, 256).transpose(2, 0, 1, 3)).astype(bf16),
        "wkp": np.ascontiguousarray(
            wk.reshape(8, 128, 4, 256).transpose(2, 0, 1, 3)).astype(bf16),
        "wv": wv.reshape(8, 128, DIN).astype(bf16),
        "wo": wo.reshape(8, 128, DOUT).astype(bf16),
        "masks": np.ascontiguousarray(masks).astype(bf16),
        "prot": prot.astype(bf16),
        "ones": np.ones((128, 1), bf16),
        "onesr": np.ones((1, 128), bf16),
        "tabs": tabs,
    }

    per_core = []
    for b in range(NCORES):
        per_core.append({
            "xT": np.ascontiguousarray(x[b].T).astype(bf16),
            "pkT": pkT[b],
            "pvf": np.ascontiguousarray(
                past_v[b].reshape(P, DIN).reshape(8, 128, DIN)).astype(bf16),
            **common,
        })
    return per_core


def build_kernel():
    import concourse.bass as bass  # noqa: F401
    import concourse.mybir as mybir
    from concourse import bacc
    from concourse.tile import TileContext

    f32 = mybir.dt.float32
    bf = mybir.dt.bfloat16
    AF = mybir.ActivationFunctionType
    OP = mybir.AluOpType

    nc = bacc.Bacc(None, target_bir_lowering=False)

    xT_d = nc.dram_tensor("xT", [DIN, T], bf, kind="ExternalInput")
    wqp_d = nc.dram_tensor("wqp", [4, 8, 128, 256], bf, kind="ExternalInput")
    wkp_d = nc.dram_tensor("wkp", [4, 8, 128, 256], bf, kind="ExternalInput")
    wv_d = nc.dram_tensor("wv", [8, 128, DIN], bf, kind="ExternalInput")
    wo_d = nc.dram_tensor("wo", [8, 128, DOUT], bf, kind="ExternalInput")
    pkT_d = nc.dram_tensor("pkT", [8, 128, P], bf, kind="ExternalInput")
    pvf_d = nc.dram_tensor("pvf", [8, 128, DIN], bf, kind="ExternalInput")
    tabs_d = nc.dram_tensor("tabs", [128, 4, T], bf, kind="ExternalInput")
    prot_d = nc.dram_tensor("prot", [128, 128], bf, kind="ExternalInput")
    masks_d = nc.dram_tensor("masks", [128, 4, 512], bf, kind="ExternalInput")
    ones_d = nc.dram_tensor("ones", [128, 1], bf, kind="ExternalInput")
    onesr_d = nc.dram_tensor("onesr", [1, 128], bf, kind="ExternalInput")
    out_d = nc.dram_tensor("out", [T, DOUT], f32, kind="ExternalOutput")

    from contextlib import ExitStack
    stack = ExitStack()
    with TileContext(nc) as tc, stack:
        cst = stack.enter_context(tc.tile_pool(name="consts", bufs=1))
        dat = stack.enter_context(tc.tile_pool(name="data", bufs=1))
        wtp = stack.enter_context(tc.tile_pool(name="wt", bufs=1))
        rawp = stack.enter_context(tc.tile_pool(name="raw", bufs=4))
        pjp = stack.enter_context(tc.tile_pool(name="pj", bufs=6))
        smlp = stack.enter_context(tc.tile_pool(name="sml", bufs=2))
        otp = stack.enter_context(tc.tile_pool(name="ot", bufs=3))
        quad = stack.enter_context(tc.tile_pool(name="quad", bufs=3, space="PSUM"))
        mxp = stack.enter_context(tc.tile_pool(name="mx", bufs=2, space="PSUM"))
        ytpp = stack.enter_context(tc.tile_pool(name="ytp", bufs=1, space="PSUM"))
        smbc = stack.enter_context(tc.tile_pool(name="smbc", bufs=1, space="PSUM"))

        # ---- persistent SBUF tiles + startup DMAs ----
        # Emission order = SP issue order; the first projection matmuls need
        # wq(h0)[kt] + xT[kt], so those lead, interleaved, with the rope
        # constants woven in. Everything else follows in order of first use.
        # xT in column halves: the first projection pass needs only the
        # th=0 half, halving the startup DMA footprint
        xTh = [[dat.tile([128, 512], bf, name=f"xT{i}_{hf}", tag=f"xT{i}_{hf}")
                for hf in range(2)] for i in range(8)]
        tabs = cst.tile([128, 4, T], bf, name="tabs", tag="tabs")
        prot = cst.tile([128, 128], bf, name="prot", tag="prot")
        ones_sb = cst.tile([128, 1], bf, name="ones", tag="ones")
        onesr_sb = cst.tile([1, 128], bf, name="onesr", tag="onesr")
        masks = cst.tile([128, 4, 512], bf, name="masks", tag="masks")
        kT = [dat.tile([128, S], bf, name=f"kT{i}", tag=f"kT{i}") for i in range(8)]
        pv_sb = [dat.tile([128, DIN], bf, name=f"pv{i}", tag=f"pv{i}")
                 for i in range(8)]
        qT = [dat.tile([128, T], bf, name=f"qT{i}", tag=f"qT{i}") for i in range(8)]

        # Startup DMA issue is spread over three queues so the projection
        # pipeline is fed at matmul pace: SP carries the q-weights (+ the
        # rest of the input stream), GpSimd carries xT/pkT, ACT the small
        # rope constants.
        # PE warmup: ~32 back-to-back matmuls on a zeroed scratch tile keep
        # the HAM clock-gate warm through the DMA-gated startup window
        scratch = wtp.tile([128, 512], bf, name="warm", tag="warm")
        nc.vector.memset(scratch[:], 0.0)
        warm_ps = quad.tile([128, 512], f32, name="warm_ps", tag="q")
        for i in range(24):
            nc.tensor.matmul(warm_ps[:], scratch[:, 0:128], scratch[:],
                             start=(i == 0), stop=(i == 23))

        wts_q0 = [wtp.tile([128, 256], bf, name="wqk", tag=f"wqk{kt}", bufs=2)
                  for kt in range(8)]
        for kt in range(8):
            nc.sync.dma_start(out=wts_q0[kt][:], in_=wqp_d[0, kt])
            nc.gpsimd.dma_start(
                out=xTh[kt][0][:], in_=xT_d[128 * kt:128 * (kt + 1), 0:512])
        for kt in range(8):
            nc.gpsimd.dma_start(
                out=xTh[kt][1][:], in_=xT_d[128 * kt:128 * (kt + 1), 512:1024])
        nc.scalar.dma_start(out=tabs[:], in_=tabs_d[:])
        nc.scalar.dma_start(out=prot[:], in_=prot_d[:])
        nc.scalar.dma_start(out=ones_sb[:], in_=ones_d[:])
        nc.scalar.dma_start(out=onesr_sb[:], in_=onesr_d[:])
        for ft in range(8):
            nc.gpsimd.dma_start(out=kT[ft][:, 0:P], in_=pkT_d[ft])
        v_sb = [dat.tile([128, DIN], bf, name=f"v{i}", tag=f"v{i}")
                for i in range(8)]
        yT = [dat.tile([128, T], bf, name=f"yT{i}", tag=f"yT{i}") for i in range(8)]

        # ---- building blocks ----
        def rope_combine(dst_ap, raw_sb, rot_ps, f2, t0):
            # dst = raw * cos + rot * sin, tables sliced at local t0
            ctab = tabs[:, f2, t0:t0 + 512]
            stab = tabs[:, 2 + f2, t0:t0 + 512]
            t1 = rawp.tile([128, 512], bf, name="ropet1", tag="ropet1")
            nc.gpsimd.tensor_tensor(t1[:], raw_sb, ctab, op=OP.mult)
            t2 = rawp.tile([128, 512], bf, name="ropet2", tag="ropet2")
            nc.vector.tensor_tensor(t2[:], rot_ps, stab, op=OP.mult)
            nc.vector.tensor_tensor(dst_ap, t1[:], t2[:], op=OP.add)

        def proj_qk(h, w_d, dst, dst_off, wts=None):
            # dst[2h+f2][:, dst_off + t] = rope(x @ w)[128f2+p, t]
            if wts is None:
                wts = [wtp.tile([128, 256], bf, name="wqk", tag=f"wqk{kt}",
                                bufs=2) for kt in range(8)]
                for kt in range(8):
                    nc.sync.dma_start(out=wts[kt][:], in_=w_d[h, kt])
            for th in range(2):
                for f2 in range(2):
                    psl = quad.tile([128, 512], f32, name="psl", tag="q")
                    for kt in range(8):
                        nc.tensor.matmul(
                            psl[:],
                            wts[kt][:, 128 * f2:128 * (f2 + 1)],
                            xTh[kt][th][:],
                            start=(kt == 0), stop=(kt == 7))
                    raw = rawp.tile([128, 512], bf, name="raw", tag="raw")
                    nc.scalar.copy(raw[:], psl[:])
                    rp = mxp.tile([128, 512], f32, name="rot", tag="mx")
                    nc.tensor.matmul(rp[:], prot[:], raw[:],
                                     start=True, stop=True)
                    rope_combine(
                        dst[2 * h + f2][:, dst_off + 512 * th:
                                        dst_off + 512 * (th + 1)],
                        raw[:], rp[:], f2, 512 * th)

        def proj_v():
            wvs = [wtp.tile([128, DIN], bf, name="wv", tag=f"wv{kt}")
                   for kt in range(8)]
            for kt in range(8):
                nc.sync.dma_start(out=wvs[kt][:], in_=wv_d[kt])
            for st in range(8):
                for fh in range(2):
                    pv_ps = quad.tile([128, 512], f32, name="pvps", tag="q")
                    for kt in range(8):
                        nc.tensor.matmul(
                            pv_ps[:],
                            xTh[kt][st // 4][:, 128 * (st % 4):
                                             128 * (st % 4 + 1)],
                            wvs[kt][:, 512 * fh:512 * (fh + 1)],
                            start=(kt == 0), stop=(kt == 7))
                    nc.vector.tensor_copy(
                        v_sb[st][:, 512 * fh:512 * (fh + 1)], pv_ps[:])

        def vsrc(j, h, fb):
            src = pv_sb[j] if j < 8 else v_sb[j - 8]
            c0 = 256 * h + 128 * fb
            return src[:, c0:c0 + 128]

        def attention(h):
            for TH in range(2):
                jmax = 12 + 4 * TH
                ytp = [ytpp.tile([128, 512], f32, name=f"ytp{i}", tag=f"y{i}")
                       for i in range(2)]
                sm = smbc.tile([128, 512], f32, name="sm", tag="sb")
                for j in range(jmax):
                    # diagonal block ci only serves local queries >= 128*ci
                    ci = j - (8 + 4 * TH)
                    q0 = 128 * ci if ci > 0 else 0
                    sc = mxp.tile([128, 512], f32, name="sc", tag="mx")
                    for fk in range(2):
                        nc.tensor.matmul(
                            sc[:, q0:512],
                            kT[2 * h + fk][:, 128 * j:128 * (j + 1)],
                            qT[2 * h + fk][:, 512 * TH + q0:512 * (TH + 1)],
                            start=(fk == 0), stop=(fk == 1))
                    pj = pjp.tile([128, 512], bf, name="pj", tag="pj")
                    nc.scalar.activation(pj[:, q0:512], sc[:, q0:512], AF.Exp,
                                         scale=float(DH ** -0.5))
                    if ci >= 0:
                        nc.vector.tensor_tensor(
                            pj[:, q0:q0 + 128], pj[:, q0:q0 + 128],
                            masks[:, ci, q0:q0 + 128], op=OP.mult)
                    for fb in range(2):
                        nc.tensor.matmul(ytp[fb][:, q0:512],
                                         vsrc(j, h, fb), pj[:, q0:512],
                                         start=(j == 0), stop=(j == jmax - 1))
                    nc.tensor.matmul(sm[0:1, q0:512], ones_sb[:],
                                     pj[:, q0:512],
                                     start=(j == 0), stop=(j == jmax - 1))
                rc = smlp.tile([1, 512], f32, name="rc", tag="rc")
                nc.vector.reciprocal_approx_fast(out=rc[:], in_=sm[0:1, :])
                rcb = smlp.tile([1, 512], bf, name="rcb", tag="rcb")
                nc.scalar.copy(rcb[:], rc[:])
                bc_ps = smbc.tile([128, 512], f32, name="bc", tag="sb")
                nc.tensor.matmul(bc_ps[:], onesr_sb[:], rcb[:],
                                 start=True, stop=True)
                bc_sb = smlp.tile([128, 512], bf, name="bcsb", tag="bcsb")
                nc.scalar.copy(bc_sb[:], bc_ps[:])
                for fb in range(2):
                    nc.vector.tensor_tensor(
                        yT[2 * h + fb][:, 512 * TH:512 * (TH + 1)],
                        ytp[fb][:], bc_sb[:], op=OP.mult)

        # ---- program ----
        proj_qk(0, wqp_d, qT, 0, wts=wts_q0)
        proj_qk(0, wkp_d, kT, P)
        proj_v()
        for st in range(8):
            nc.sync.dma_start(out=pv_sb[st][:], in_=pvf_d[st])
        nc.sync.dma_start(out=masks[:], in_=masks_d[:])
        attention(0)
        wo_sb = [dat.tile([128, DOUT], bf, name=f"wo{i}", tag=f"wo{i}")
                 for i in range(8)]
        for h in range(1, 4):
            proj_qk(h, wqp_d, qT, 0)
            proj_qk(h, wkp_d, kT, P)
            if h == 2:
                for kt in range(8):
                    nc.sync.dma_start(out=wo_sb[kt][:], in_=wo_d[kt])
            attention(h)

        for tt in range(8):
            for ds in range(3):
                op_ps = quad.tile([128, 512], f32, name="ops", tag="q")
                for fk in range(8):
                    nc.tensor.matmul(
                        op_ps[:, 0:384],
                        yT[fk][:, 128 * tt:128 * (tt + 1)],
                        wo_sb[fk][:, 384 * ds:384 * (ds + 1)],
                        start=(fk == 0), stop=(fk == 7))
                ot = otp.tile([128, 384], f32, name="ot", tag="ot")
                nc.vector.tensor_copy(ot[:], op_ps[:, 0:384])
                nc.sync.dma_start(
                    out=out_d[128 * tt:128 * (tt + 1), 384 * ds:384 * (ds + 1)],
                    in_=ot[:])

    nc.finalize()
    return nc


_NC_CACHE = {}


def run(x, past_k, past_v, wq, wk, wv, wo, debug=False, trace=False):
    from concourse.bass_utils import run_bass_kernel_spmd

    if "nc" not in _NC_CACHE:
        _NC_CACHE["nc"] = build_kernel()
    nc = _NC_CACHE["nc"]
    in_maps = _host_prep(x, past_k, past_v, wq, wk, wv, wo)
    res = run_bass_kernel_spmd(nc, in_maps, list(range(NCORES)), trace=trace)
    out = np.stack([res.results[b]["out"] for b in range(NCORES)], axis=0)
    return out.astype(np.float32), res


def kernel(x, past_k, past_v, wq, wk, wv, wo):
    out, _ = run(x, past_k, past_v, wq, wk, wv, wo)
    return out


# revision 34
# speedup vs baseline: 1.2423x; 1.0748x over previous
"""Bass/Trainium2 kernel for nn_BakaAttention: 8-way data-parallel over batch.

Per core (one batch element):
  q = rope(x@wq, off=1024); k = concat(rope_host(past_k), rope(x@wk));
  v = concat(past_v, x@wv); out = softmax(mask(q k^T / 16)) v @ wo

All matmuls run in bf16 (1 cycle/row on the PE vs 2 for f32r's fp32-HIGH
mode). x is pre-transposed and past_k pre-roped+transposed on the host, so
the device does no transposes. Everything stays SBUF-resident (no DRAM
spills). Scores are computed transposed [keys, queries] so PV consumes the
probs directly as the moving operand; softmax row-sums accumulate via a
ones-column matmul and the normalization uses the fast custom-DVE
reciprocal off the critical path.
"""

import numpy as np

B, T, P, H, DH, DIN, DOUT = 8, 1024, 1024, 4, 256, 1024, 1152
S = P + T  # 2048 keys
THETA = 10000.0
NCORES = 8


def _host_prep(x, past_k, past_v, wq, wk, wv, wo):
    """Per-batch input prep: bf16 casts, transposes, past_k rope."""
    import ml_dtypes

    bf16 = ml_dtypes.bfloat16
    x, past_k, past_v, wq, wk, wv, wo = (
        np.asarray(a) for a in (x, past_k, past_v, wq, wk, wv, wo))

    m = np.arange(0, DH, 2, dtype=np.float64) / DH          # 128 freqs
    inv = 1.0 / (THETA ** m)                                # [128]

    # past_k rope at offset 0, interleaved pairs
    pos = np.arange(P, dtype=np.float64)
    ang = np.outer(pos, inv)                                # [P, 128]
    c = np.cos(ang)[:, None, :]                             # [P, 1, 128]
    s = np.sin(ang)[:, None, :]
    pk = past_k.astype(np.float64)                          # [B, P, H, DH]
    x1, x2 = pk[..., 0::2], pk[..., 1::2]
    o1 = x1 * c - x2 * s
    o2 = x2 * c + x1 * s
    pk_rot = np.stack([o1, o2], axis=-1).reshape(B, P, H, DH)
    # kT layout: [B, 8 ftile, 128, P]; ftile ft=2h+f2 covers head-local
    # features 128*f2 + p
    pkT = np.ascontiguousarray(
        pk_rot.reshape(B, P, 8, 128).transpose(0, 2, 3, 1)
    ).astype(bf16)

    # device rope tables for positions P..P+T-1 (used by both q and new-k)
    posq = np.arange(P, P + T, dtype=np.float64)
    angq = np.outer(inv, posq)                              # [128 m, T]
    cosq, sinq = np.cos(angq), np.sin(angq)
    r = np.arange(128) // 2
    tabs = np.ascontiguousarray(np.stack(
        [cosq[r, :], cosq[64 + r, :], sinq[r, :], sinq[64 + r, :]],
        axis=1)).astype(bf16)  # [128, 4, T]

    # pair-rotation matrix: rot = prot.T @ raw -> rot[2m] = -raw[2m+1],
    # rot[2m+1] = raw[2m]
    prot = np.zeros((128, 128), np.float32)
    for mm in range(64):
        prot[2 * mm, 2 * mm + 1] = 1.0
        prot[2 * mm + 1, 2 * mm] = -1.0

    # masks[p, ci, tl] = 1 if key p within diag block ci is visible to
    # local query tl
    sl = np.arange(128)[:, None]
    tl = np.arange(512)[None, :]
    masks = np.stack(
        [(sl <= tl - 128 * ci).astype(np.float32) for ci in range(4)], axis=1
    )

    common = {
        "wqp": np.ascontiguousarray(
            wq.reshape(8, 128, 4, 256).transpose(2, 0, 1, 3)).astype(bf16),
        "wkp": np.ascontiguousarray(
            wk.reshape(8, 128, 4, 256).transpose(2, 0, 1, 3)).astype(bf16),
        "wv": wv.reshape(8, 128, DIN).astype(bf16),
        "wo": wo.reshape(8, 128, DOUT).astype(bf16),
        "masks": np.ascontiguousarray(masks).astype(bf16),
        "prot": prot.astype(bf16),
        "ones": np.ones((128, 1), bf16),
        "onesr": np.ones((1, 128), bf16),
        "tabs": tabs,
    }

    per_core = []
    for b in range(NCORES):
        per_core.append({
            "xT": np.ascontiguousarray(x[b].T).astype(bf16),
            "pkT": pkT[b],
            "pvf": np.ascontiguousarray(
                past_v[b].reshape(P, DIN).reshape(8, 128, DIN)).astype(bf16),
            **common,
        })
    return per_core


def build_kernel():
    import concourse.bass as bass  # noqa: F401
    import concourse.mybir as mybir
    from concourse import bacc
    from concourse.tile import TileContext

    f32 = mybir.dt.float32
    bf = mybir.dt.bfloat16
    AF = mybir.ActivationFunctionType
    OP = mybir.AluOpType

    nc = bacc.Bacc(None, target_bir_lowering=False)

    xT_d = nc.dram_tensor("xT", [DIN, T], bf, kind="ExternalInput")
    wqp_d = nc.dram_tensor("wqp", [4, 8, 128, 256], bf, kind="ExternalInput")
    wkp_d = nc.dram_tensor("wkp", [4, 8, 128, 256], bf, kind="ExternalInput")
    wv_d = nc.dram_tensor("wv", [8, 128, DIN], bf, kind="ExternalInput")
    wo_d = nc.dram_tensor("wo", [8, 128, DOUT], bf, kind="ExternalInput")
    pkT_d = nc.dram_tensor("pkT", [8, 128, P], bf, kind="ExternalInput")
    pvf_d = nc.dram_tensor("pvf", [8, 128, DIN], bf, kind="ExternalInput")
    tabs_d = nc.dram_tensor("tabs", [128, 4, T], bf, kind="ExternalInput")
    prot_d = nc.dram_tensor("prot", [128, 128], bf, kind="ExternalInput")
    masks_d = nc.dram_tensor("masks", [128, 4, 512], bf, kind="ExternalInput")
    ones_d = nc.dram_tensor("ones", [128, 1], bf, kind="ExternalInput")
    onesr_d = nc.dram_tensor("onesr", [1, 128], bf, kind="ExternalInput")
    out_d = nc.dram_tensor("out", [T, DOUT], f32, kind="ExternalOutput")

    from contextlib import ExitStack
    stack = ExitStack()
    with TileContext(nc) as tc, stack:
        cst = stack.enter_context(tc.tile_pool(name="consts", bufs=1))
        dat = stack.enter_context(tc.tile_pool(name="data", bufs=1))
        wtp = stack.enter_context(tc.tile_pool(name="wt", bufs=1))
        rawp = stack.enter_context(tc.tile_pool(name="raw", bufs=3))
        pjp = stack.enter_context(tc.tile_pool(name="pj", bufs=6))
        smlp = stack.enter_context(tc.tile_pool(name="sml", bufs=2))
        otp = stack.enter_context(tc.tile_pool(name="ot", bufs=2))
        quad = stack.enter_context(tc.tile_pool(name="quad", bufs=3, space="PSUM"))
        mxp = stack.enter_context(tc.tile_pool(name="mx", bufs=2, space="PSUM"))
        ytpp = stack.enter_context(tc.tile_pool(name="ytp", bufs=1, space="PSUM"))
        smbc = stack.enter_context(tc.tile_pool(name="smbc", bufs=1, space="PSUM"))

        # ---- persistent SBUF tiles + startup DMAs ----
        # Emission order = SP issue order; the first projection matmuls need
        # wq(h0)[kt] + xT[kt], so those lead, interleaved, with the rope
        # constants woven in. Everything else follows in order of first use.
        # xT in column halves: the first projection pass needs only the
        # th=0 half, halving the startup DMA footprint
        xTh = [[dat.tile([128, 512], bf, name=f"xT{i}_{hf}", tag=f"xT{i}_{hf}")
                for hf in range(2)] for i in range(8)]
        tabs = cst.tile([128, 4, T], bf, name="tabs", tag="tabs")
        prot = cst.tile([128, 128], bf, name="prot", tag="prot")
        ones_sb = cst.tile([128, 1], bf, name="ones", tag="ones")
        onesr_sb = cst.tile([1, 128], bf, name="onesr", tag="onesr")
        masks = cst.tile([128, 4, 512], bf, name="masks", tag="masks")
        kT = [dat.tile([128, S], bf, name=f"kT{i}", tag=f"kT{i}") for i in range(8)]
        pv_sb = [dat.tile([128, DIN], bf, name=f"pv{i}", tag=f"pv{i}")
                 for i in range(8)]
        qT = [dat.tile([128, T], bf, name=f"qT{i}", tag=f"qT{i}") for i in range(8)]

        # Startup DMA issue is spread over three queues so the projection
        # pipeline is fed at matmul pace: SP carries the q-weights (+ the
        # rest of the input stream), GpSimd carries xT/pkT, ACT the small
        # rope constants.
        # PE warmup: ~32 back-to-back matmuls on a zeroed scratch tile keep
        # the HAM clock-gate warm through the DMA-gated startup window
        scratch = wtp.tile([128, 512], bf, name="warm", tag="warm")
        nc.vector.memset(scratch[:], 0.0)
        warm_ps = quad.tile([128, 512], f32, name="warm_ps", tag="q")
        for i in range(24):
            nc.tensor.matmul(warm_ps[:], scratch[:, 0:128], scratch[:],
                             start=(i == 0), stop=(i == 23))

        wts_q0 = [wtp.tile([128, 256], bf, name="wqk", tag=f"wqk{kt}", bufs=2)
                  for kt in range(8)]
        for kt in range(8):
            nc.sync.dma_start(out=wts_q0[kt][:], in_=wqp_d[0, kt])
            nc.gpsimd.dma_start(
                out=xTh[kt][0][:], in_=xT_d[128 * kt:128 * (kt + 1), 0:512])
        for kt in range(8):
            nc.gpsimd.dma_start(
                out=xTh[kt][1][:], in_=xT_d[128 * kt:128 * (kt + 1), 512:1024])
        nc.scalar.dma_start(out=tabs[:], in_=tabs_d[:])
        nc.scalar.dma_start(out=prot[:], in_=prot_d[:])
        nc.scalar.dma_start(out=ones_sb[:], in_=ones_d[:])
        nc.scalar.dma_start(out=onesr_sb[:], in_=onesr_d[:])
        for ft in range(8):
            nc.gpsimd.dma_start(out=kT[ft][:, 0:P], in_=pkT_d[ft])
        v_sb = [dat.tile([128, DIN], bf, name=f"v{i}", tag=f"v{i}")
                for i in range(8)]
        yT = [dat.tile([128, T], bf, name=f"yT{i}", tag=f"yT{i}") for i in range(8)]

        # ---- building blocks ----
        def rope_combine(dst_ap, raw_sb, rot_ps, f2, t0):
            # dst = raw * cos + rot * sin, tables sliced at local t0
            ctab = tabs[:, f2, t0:t0 + 512]
            stab = tabs[:, 2 + f2, t0:t0 + 512]
            t1 = rawp.tile([128, 512], bf, name="ropet1", tag="ropet1")
            nc.gpsimd.tensor_tensor(t1[:], raw_sb, ctab, op=OP.mult)
            t2 = rawp.tile([128, 512], bf, name="ropet2", tag="ropet2")
            nc.vector.tensor_tensor(t2[:], rot_ps, stab, op=OP.mult)
            nc.vector.tensor_tensor(dst_ap, t1[:], t2[:], op=OP.add)

        def proj_qk(h, w_d, dst, dst_off, wts=None):
            # dst[2h+f2][:, dst_off + t] = rope(x @ w)[128f2+p, t]
            if wts is None:
                wts = [wtp.tile([128, 256], bf, name="wqk", tag=f"wqk{kt}",
                                bufs=2) for kt in range(8)]
                for kt in range(8):
                    nc.sync.dma_start(out=wts[kt][:], in_=w_d[h, kt])
            for th in range(2):
                for f2 in range(2):
                    psl = quad.tile([128, 512], f32, name="psl", tag="q")
                    for kt in range(8):
                        nc.tensor.matmul(
                            psl[:],
                            wts[kt][:, 128 * f2:128 * (f2 + 1)],
                            xTh[kt][th][:],
                            start=(kt == 0), stop=(kt == 7))
                    raw = rawp.tile([128, 512], bf, name="raw", tag="raw")
                    nc.scalar.copy(raw[:], psl[:])
                    rp = mxp.tile([128, 512], f32, name="rot", tag="mx")
                    nc.tensor.matmul(rp[:], prot[:], raw[:],
                                     start=True, stop=True)
                    rope_combine(
                        dst[2 * h + f2][:, dst_off + 512 * th:
                                        dst_off + 512 * (th + 1)],
                        raw[:], rp[:], f2, 512 * th)

        def proj_v():
            wvs = [wtp.tile([128, DIN], bf, name="wv", tag=f"wv{kt}")
                   for kt in range(8)]
            for kt in range(8):
                nc.sync.dma_start(out=wvs[kt][:], in_=wv_d[kt])
            for st in range(8):
                for fh in range(2):
                    pv_ps = quad.tile([128, 512], f32, name="pvps", tag="q")
                    for kt in range(8):
                        nc.tensor.matmul(
                            pv_ps[:],
                            xTh[kt][st // 4][:, 128 * (st % 4):
                                             128 * (st % 4 + 1)],
                            wvs[kt][:, 512 * fh:512 * (fh + 1)],
                            start=(kt == 0), stop=(kt == 7))
                    nc.vector.tensor_copy(
                        v_sb[st][:, 512 * fh:512 * (fh + 1)], pv_ps[:])

        def vsrc(j, h, fb):
            src = pv_sb[j] if j < 8 else v_sb[j - 8]
            c0 = 256 * h + 128 * fb
            return src[:, c0:c0 + 128]

        def attention(h):
            for TH in range(2):
                jmax = 12 + 4 * TH
                ytp = [ytpp.tile([128, 512], f32, name=f"ytp{i}", tag=f"y{i}")
                       for i in range(2)]
                sm = smbc.tile([128, 512], f32, name="sm", tag="sb")
                # rowsums for full blocks are quad-batched: 4 prob tiles
                # pre-summed on DVE, one ones-matmul per quad
                fulls = []
                started = [False]

                def rowsum(ap, q0, last):
                    nc.tensor.matmul(sm[0:1, q0:512], ones_sb[:], ap,
                                     start=(not started[0]), stop=last)
                    started[0] = True

                for j in range(jmax):
                    # diagonal block ci only serves local queries >= 128*ci
                    ci = j - (8 + 4 * TH)
                    q0 = 128 * ci if ci > 0 else 0
                    sc = mxp.tile([128, 512], f32, name="sc", tag="mx")
                    for fk in range(2):
                        nc.tensor.matmul(
                            sc[:, q0:512],
                            kT[2 * h + fk][:, 128 * j:128 * (j + 1)],
                            qT[2 * h + fk][:, 512 * TH + q0:512 * (TH + 1)],
                            start=(fk == 0), stop=(fk == 1))
                    pj = pjp.tile([128, 512], bf, name="pj", tag="pj", bufs=8)
                    nc.scalar.activation(pj[:, q0:512], sc[:, q0:512], AF.Exp,
                                         scale=float(DH ** -0.5))
                    if ci >= 0:
                        nc.vector.tensor_tensor(
                            pj[:, q0:q0 + 128], pj[:, q0:q0 + 128],
                            masks[:, ci, q0:q0 + 128], op=OP.mult)
                    for fb in range(2):
                        nc.tensor.matmul(ytp[fb][:, q0:512],
                                         vsrc(j, h, fb), pj[:, q0:512],
                                         start=(j == 0), stop=(j == jmax - 1))
                    if ci < 0:
                        fulls.append(pj)
                        if len(fulls) == 4:
                            pa = pjp.tile([128, 512], bf, name="padd",
                                          tag="padd", bufs=2)
                            nc.vector.tensor_tensor(pa[:], fulls[0][:],
                                                    fulls[1][:], op=OP.add)
                            pb = pjp.tile([128, 512], bf, name="padd2",
                                          tag="padd2", bufs=2)
                            nc.vector.tensor_tensor(pb[:], fulls[2][:],
                                                    fulls[3][:], op=OP.add)
                            pc = pjp.tile([128, 512], bf, name="padd3",
                                          tag="padd3", bufs=2)
                            nc.vector.tensor_tensor(pc[:], pa[:], pb[:],
                                                    op=OP.add)
                            rowsum(pc[:], 0, False)
                            fulls.clear()
                    else:
                        rowsum(pj[:, q0:512], q0, j == jmax - 1)
                rc = smlp.tile([1, 512], f32, name="rc", tag="rc")
                nc.vector.reciprocal_approx_fast(out=rc[:], in_=sm[0:1, :])
                rcb = smlp.tile([1, 512], bf, name="rcb", tag="rcb")
                nc.scalar.copy(rcb[:], rc[:])
                bc_ps = smbc.tile([128, 512], f32, name="bc", tag="sb")
                nc.tensor.matmul(bc_ps[:], onesr_sb[:], rcb[:],
                                 start=True, stop=True)
                bc_sb = smlp.tile([128, 512], bf, name="bcsb", tag="bcsb")
                nc.scalar.copy(bc_sb[:], bc_ps[:])
                for fb in range(2):
                    nc.vector.tensor_tensor(
                        yT[2 * h + fb][:, 512 * TH:512 * (TH + 1)],
                        ytp[fb][:], bc_sb[:], op=OP.mult)

        # ---- program ----
        proj_qk(0, wqp_d, qT, 0, wts=wts_q0)
        proj_qk(0, wkp_d, kT, P)
        proj_v()
        for st in range(8):
            nc.sync.dma_start(out=pv_sb[st][:], in_=pvf_d[st])
        nc.sync.dma_start(out=masks[:], in_=masks_d[:])
        attention(0)
        wo_sb = [dat.tile([128, DOUT], bf, name=f"wo{i}", tag=f"wo{i}")
                 for i in range(8)]
        for h in range(1, 4):
            proj_qk(h, wqp_d, qT, 0)
            proj_qk(h, wkp_d, kT, P)
            if h == 2:
                for kt in range(8):
                    nc.sync.dma_start(out=wo_sb[kt][:], in_=wo_d[kt])
            attention(h)

        for tt in range(8):
            for ds in range(3):
                op_ps = quad.tile([128, 512], f32, name="ops", tag="q")
                for fk in range(8):
                    nc.tensor.matmul(
                        op_ps[:, 0:384],
                        yT[fk][:, 128 * tt:128 * (tt + 1)],
                        wo_sb[fk][:, 384 * ds:384 * (ds + 1)],
                        start=(fk == 0), stop=(fk == 7))
                ot = otp.tile([128, 384], f32, name="ot", tag="ot")
                nc.vector.tensor_copy(ot[:], op_ps[:, 0:384])
                nc.sync.dma_start(
                    out=out_d[128 * tt:128 * (tt + 1), 384 * ds:384 * (ds + 1)],
                    in_=ot[:])

    nc.finalize()
    return nc


_NC_CACHE = {}


def run(x, past_k, past_v, wq, wk, wv, wo, debug=False, trace=False):
    from concourse.bass_utils import run_bass_kernel_spmd

    if "nc" not in _NC_CACHE:
        _NC_CACHE["nc"] = build_kernel()
    nc = _NC_CACHE["nc"]
    in_maps = _host_prep(x, past_k, past_v, wq, wk, wv, wo)
    res = run_bass_kernel_spmd(nc, in_maps, list(range(NCORES)), trace=trace)
    out = np.stack([res.results[b]["out"] for b in range(NCORES)], axis=0)
    return out.astype(np.float32), res


def kernel(x, past_k, past_v, wq, wk, wv, wo):
    out, _ = run(x, past_k, past_v, wq, wk, wv, wo)
    return out
